# revision 4
# baseline (speedup 1.0000x reference)
"""Trainium2 Bass kernel for nn_PersistentObserver (GNN message passing).

Math (per batch item b, N=256 nodes):
  node_emb = relu(relu(obs@W1+b1)@W2+b2)            [N,256]
  upd      = node_emb@Wu+bu                         [N,128]
  lat      = GRUCell(upd, latent)                   [N,128]
  pair_ij  = [lat_i, lat_j, |lat_i-lat_j|]          [N,N,384]
  edge     = relu(pair@We1+be1)@We2+be2, diag=-8    [N,N]
  op       = relu(pair@Wo1+bo1)@Wo2+bo2             [N,N,8]
  next     = relu([lat,node_emb,q_emb]@Wn1+bn1)@Wn2+bn2  [N,1]

Key factorization: pair@W1 = A_i + B_j + |lat_i-lat_j|@W1a where
A = lat@W1_left, B = lat@W1_right depend on one index only. Only the
abs-diff term needs the N^2 matmul.

Sharding: 8 cores = 4 batches x 2 row-halves. Odd cores receive inputs
rolled by -128 along the node axis (the computation is permutation
equivariant), so every core runs the identical program computing rows
0..127; the host un-rolls the j axis on gather.

Layout: everything transposed ("T"): features on partitions, nodes on
the free axis. Hidden H=256 is split in two partition halves. The
per-2-rows inner loop (64 iterations):
  ACT : absd[:,ii*256:+256] = |(-latn) + latn_col(i)|   (f32r out)
  PE  : psum_h = W1a_half^T @ absd  (+= W1r_half^T @ [latn|latn])
  ACT/DVE: hT = relu(psum_h + (A_i+b1) col)             (f32r out)
  PE  : edge psum[1,512] = We2^T @ hT_e ;  op psum[8,512] = Wo2^T @ hT_o
  ACT/DVE: out = psum + b2 ; gpsimd memset diag=-8 ; DMA out
"""
import numpy as np
from contextlib import ExitStack

import concourse.bass as bass
import concourse.tile as tile
from concourse import bacc, mybir
from concourse.bass_utils import run_bass_kernel_spmd

F32 = mybir.dt.float32
F32R = mybir.dt.float32r
A = mybir.AluOpType
AF = mybir.ActivationFunctionType

B, N, OBS, QDIM = 4, 256, 64, 32
H, D = 256, 128
NOPS = 8
QE = 64          # H // 4
NI = 128         # i-rows per core
N_CORES = 8

_NC_CACHE = {}


def _build():
    nc = bacc.Bacc("TRN2", target_bir_lowering=False, debug=False,
                   num_devices=N_CORES)

    di = {}
    def inp(name, shape):
        di[name] = nc.dram_tensor(name, list(shape), F32, kind="ExternalInput").ap()
        return di[name]

    inp("obs", (N, OBS)); inp("latent", (N, D)); inp("query", (N, QDIM))
    inp("enc_w1", (OBS, H)); inp("enc_b1", (H,))
    inp("enc_w2", (H, H)); inp("enc_b2", (H,))
    inp("upd_w", (H, D)); inp("upd_b", (D,))
    inp("gru_wi", (D, 3 * D)); inp("gru_bi", (3 * D,))
    inp("gru_wh", (D, 3 * D)); inp("gru_bh", (3 * D,))
    inp("eh_w1", (3 * D, H)); inp("eh_b1", (H,))
    inp("eh_w2", (H, 1)); inp("eh_b2", (1,))
    inp("oh_w1", (3 * D, H)); inp("oh_b1", (H,))
    inp("oh_w2", (H, NOPS)); inp("oh_b2", (NOPS,))
    inp("q_w", (QDIM, QE)); inp("q_b", (QE,))
    inp("nh_w1", (D + H + QE, H)); inp("nh_b1", (H,))
    inp("nh_w2", (H, 1)); inp("nh_b2", (1,))

    edge_d = nc.dram_tensor("edge_out", [NI, N], F32, kind="ExternalOutput").ap()
    op_d = nc.dram_tensor("op_out", [NI * NOPS, N], F32, kind="ExternalOutput").ap()
    np_d = nc.dram_tensor("np_out", [N], F32, kind="ExternalOutput").ap()

    with tile.TileContext(nc) as tc, ExitStack() as ctx:
        cst = ctx.enter_context(tc.tile_pool(name="cst", bufs=1))
        act = ctx.enter_context(tc.tile_pool(name="act", bufs=1))
        wrk = ctx.enter_context(tc.tile_pool(name="wrk", bufs=2))
        ps1 = ctx.enter_context(tc.tile_pool(name="ps1", bufs=1, space="PSUM"))
        ps2 = ctx.enter_context(tc.tile_pool(name="ps2", bufs=2, space="PSUM"))

        # ---------------- weight / bias loads ----------------
        def load(name, view, shape, dt=F32):
            t = cst.tile(list(shape), dt, tag=name)
            src = view if dt == F32 else view.bitcast(F32R)
            nc.sync.dma_start(t[:], src)
            return t

        ew1 = load("ew1", di["enc_w1"][:], (OBS, H))
        eb1 = load("eb1", di["enc_b1"].rearrange("(s p) -> p s", s=2), (128, 2))
        ew2 = load("ew2", di["enc_w2"].rearrange("(kh kl) m -> kl kh m", kh=2), (128, 512))
        eb2 = load("eb2", di["enc_b2"].rearrange("(s p) -> p s", s=2), (128, 2))
        uw = load("uw", di["upd_w"].rearrange("(kh kl) m -> kl kh m", kh=2), (128, 256))
        ub = load("ub", di["upd_b"].rearrange("(p o) -> p o", o=1), (128, 1))
        gwi = load("gwi", di["gru_wi"][:], (128, 384))
        gbi = load("gbi", di["gru_bi"].rearrange("(s p) -> p s", s=3), (128, 3))
        gwh = load("gwh", di["gru_wh"][:], (128, 384))
        gbh = load("gbh", di["gru_bh"].rearrange("(s p) -> p s", s=3), (128, 3))
        hw1e = load("hw1e", di["eh_w1"].rearrange("(s kl) m -> kl s m", s=3), (128, 768), F32R)
        hb1e = load("hb1e", di["eh_b1"].rearrange("(s p) -> p s", s=2), (128, 2))
        hw2e = load("hw2e", di["eh_w2"].rearrange("(s kl) m -> kl s m", s=2), (128, 2), F32R)
        hb2e = load("hb2e", di["eh_b2"].rearrange("(p o) -> p o", o=1), (1, 1))
        hw1o = load("hw1o", di["oh_w1"].rearrange("(s kl) m -> kl s m", s=3), (128, 768), F32R)
        hb1o = load("hb1o", di["oh_b1"].rearrange("(s p) -> p s", s=2), (128, 2))
        hw2o = load("hw2o", di["oh_w2"].rearrange("(s kl) m -> kl s m", s=2), (128, 16), F32R)
        hb2o = load("hb2o", di["oh_b2"].rearrange("(p o) -> p o", o=1), (NOPS, 1))
        qw = load("qw", di["q_w"][:], (QDIM, QE))
        qb = load("qb", di["q_b"].rearrange("(p o) -> p o", o=1), (QE, 1))
        nw1a = load("nw1a", di["nh_w1"][0:384, :].rearrange("(s kl) m -> kl s m", s=3), (128, 768))
        nw1b = load("nw1b", di["nh_w1"][384:448, :], (QE, H))
        nb1 = load("nb1", di["nh_b1"].rearrange("(s p) -> p s", s=2), (128, 2))
        nw2 = load("nw2", di["nh_w2"].rearrange("(s kl) m -> kl s m", s=2), (128, 2))
        nb2 = load("nb2", di["nh_b2"].rearrange("(p o) -> p o", o=1), (1, 1))

        # transposed activations (strided DMA, exact fp32)
        obsT = cst.tile([OBS, N], F32, tag="obsT")
        nc.sync.dma_start(obsT[:], di["obs"].rearrange("n f -> f n"))
        latT = cst.tile([D, N], F32, tag="latT")
        nc.sync.dma_start(latT[:], di["latent"].rearrange("n d -> d n"))
        qT = cst.tile([QDIM, N], F32, tag="qT")
        nc.sync.dma_start(qT[:], di["query"].rearrange("n q -> q n"))

        # ---------------- per-batch precompute ----------------
        PC_TAGS = ["he0", "he1", "ho0", "ho1"]
        pc_i = [0]
        def pc_psum(p_dim, f_dim):
            t = ps1.tile([p_dim, f_dim], F32, tag=PC_TAGS[pc_i[0] % 4])
            pc_i[0] += 1
            return t

        # encoder layer 1: h1T[hh] = relu(W1[:,hh]^T @ obsT + b1)
        h1T = act.tile([128, 512], F32, tag="h1T")
        for hh in range(2):
            ps = pc_psum(128, N)
            nc.tensor.matmul(ps[:], ew1[:, hh * 128:(hh + 1) * 128], obsT[:],
                             start=True, stop=True)
            nc.scalar.activation(h1T[:, hh * 256:(hh + 1) * 256], ps[:],
                                 AF.Relu, bias=eb1[:, hh:hh + 1])
        # encoder layer 2
        nembT = act.tile([128, 512], F32, tag="nembT")
        for hh in range(2):
            ps = pc_psum(128, N)
            for kh in range(2):
                nc.tensor.matmul(ps[:], ew2[:, kh * 256 + hh * 128: kh * 256 + (hh + 1) * 128],
                                 h1T[:, kh * 256:(kh + 1) * 256],
                                 start=(kh == 0), stop=(kh == 1))
            nc.scalar.activation(nembT[:, hh * 256:(hh + 1) * 256], ps[:],
                                 AF.Relu, bias=eb2[:, hh:hh + 1])
        # upd head
        updT = act.tile([D, N], F32, tag="updT")
        ps = pc_psum(D, N)
        for kh in range(2):
            nc.tensor.matmul(ps[:], uw[:, kh * 128:(kh + 1) * 128],
                             nembT[:, kh * 256:(kh + 1) * 256],
                             start=(kh == 0), stop=(kh == 1))
        nc.scalar.activation(updT[:], ps[:], AF.Identity, bias=ub[:])
        # GRU gates
        giT = act.tile([D, 768], F32, tag="giT")
        ghT = act.tile([D, 768], F32, tag="ghT")
        for g in range(3):
            ps = pc_psum(D, N)
            nc.tensor.matmul(ps[:], gwi[:, g * 128:(g + 1) * 128], updT[:],
                             start=True, stop=True)
            nc.scalar.activation(giT[:, g * 256:(g + 1) * 256], ps[:],
                                 AF.Identity, bias=gbi[:, g:g + 1])
            ps = pc_psum(D, N)
            nc.tensor.matmul(ps[:], gwh[:, g * 128:(g + 1) * 128], latT[:],
                             start=True, stop=True)
            nc.scalar.activation(ghT[:, g * 256:(g + 1) * 256], ps[:],
                                 AF.Identity, bias=gbh[:, g:g + 1])
        rT = act.tile([D, N], F32, tag="rT")
        nc.vector.tensor_add(rT[:], giT[:, 0:256], ghT[:, 0:256])
        nc.scalar.activation(rT[:], rT[:], AF.Sigmoid)
        zT = act.tile([D, N], F32, tag="zT")
        nc.vector.tensor_add(zT[:], giT[:, 256:512], ghT[:, 256:512])
        nc.scalar.activation(zT[:], zT[:], AF.Sigmoid)
        nT = act.tile([D, N], F32, tag="nT")
        nc.vector.tensor_mul(nT[:], rT[:], ghT[:, 512:768])
        nc.vector.tensor_add(nT[:], nT[:], giT[:, 512:768])
        nc.scalar.activation(nT[:], nT[:], AF.Tanh)
        # latn = n + z*(lat - n)
        latn = act.tile([D, N], F32, tag="latn")
        nc.vector.tensor_sub(latn[:], latT[:], nT[:])
        nc.vector.tensor_mul(latn[:], zT[:], latn[:])
        nc.vector.tensor_add(latn[:], latn[:], nT[:])
        # duplicated f32r copy [latn | latn] for the per-i B accumulation
        latn2 = act.tile([D, 512], F32R, tag="latn2")
        nc.vector.tensor_scalar(latn2[:, 0:256], latn[:], 0.0, None, op0=A.add)
        nc.vector.tensor_scalar(latn2[:, 256:512], latn[:], 0.0, None, op0=A.add)

        # A_i + b1 tables (bias columns for the relu epilogue)
        ABe = act.tile([128, 512], F32, tag="ABe")
        ABo = act.tile([128, 512], F32, tag="ABo")
        for (ab, w1, b1) in ((ABe, hw1e, hb1e), (ABo, hw1o, hb1o)):
            for hh in range(2):
                ps = pc_psum(128, N)
                nc.tensor.matmul(ps[:], w1[:, 0 * 256 + hh * 128: 0 * 256 + (hh + 1) * 128],
                                 latn2[:, 0:256], start=True, stop=True)
                nc.scalar.activation(ab[:, hh * 256:(hh + 1) * 256], ps[:],
                                     AF.Identity, bias=b1[:, hh:hh + 1])

        # query encoder + next_pred head
        qeT = act.tile([QE, N], F32, tag="qeT")
        ps = pc_psum(QE, N)
        nc.tensor.matmul(ps[:], qw[:], qT[:], start=True, stop=True)
        nc.scalar.activation(qeT[:], ps[:], AF.Relu, bias=qb[:])
        nh1T = act.tile([128, 512], F32, tag="nh1T")
        for hh in range(2):
            ps = pc_psum(128, N)
            nc.tensor.matmul(ps[:], nw1a[:, 0 * 256 + hh * 128: (0 * 256) + (hh + 1) * 128],
                             latn[:], start=True, stop=False)
            nc.tensor.matmul(ps[:], nw1a[:, 1 * 256 + hh * 128: (1 * 256) + (hh + 1) * 128],
                             nembT[:, 0:256], start=False, stop=False)
            nc.tensor.matmul(ps[:], nw1a[:, 2 * 256 + hh * 128: (2 * 256) + (hh + 1) * 128],
                             nembT[:, 256:512], start=False, stop=False)
            nc.tensor.matmul(ps[:], nw1b[:, hh * 128:(hh + 1) * 128], qeT[:],
                             start=False, stop=True)
            nc.scalar.activation(nh1T[:, hh * 256:(hh + 1) * 256], ps[:],
                                 AF.Relu, bias=nb1[:, hh:hh + 1])
        ps = ps2.tile([1, N], F32, tag="pe")
        for hh in range(2):
            nc.tensor.matmul(ps[:], nw2[:, hh:hh + 1], nh1T[:, hh * 256:(hh + 1) * 256],
                             start=(hh == 0), stop=(hh == 1))
        np_sb = wrk.tile([1, N], F32, tag="np_sb")
        nc.vector.tensor_scalar(np_sb[:], ps[:], nb2[0:1, 0:1], None, op0=A.add)
        nc.sync.dma_start(np_d.rearrange("(a b) -> a b", a=1), np_sb[:])

        # ---------------- main pair loop: 64 x (2 rows) ----------------
        # epilogue engine split: 3 on ACT, 5 on DVE (see module docstring)
        EPI_ACT = {(0, 0, 0), (0, 1, 0), (1, 0, 0)}
        for ip in range(NI // 2):
            i0 = 2 * ip
            absd = wrk.tile([128, 512], F32R, tag="absd")
            for ii in range(2):
                nc.scalar.activation(absd[:, ii * 256:(ii + 1) * 256], latn[:],
                                     AF.Abs, bias=latn[:, i0 + ii:i0 + ii + 1],
                                     scale=-1.0)
            hts = {}
            for hd, (w1, ab) in enumerate(((hw1e, ABe), (hw1o, ABo))):
                for hh in range(2):
                    ps = ps1.tile([128, 512], F32, tag=PC_TAGS[hd * 2 + hh])
                    nc.tensor.matmul(ps[:], w1[:, 2 * 256 + hh * 128: 2 * 256 + (hh + 1) * 128],
                                     absd[:], start=True, stop=False)
                    nc.tensor.matmul(ps[:], w1[:, 1 * 256 + hh * 128: 1 * 256 + (hh + 1) * 128],
                                     latn2[:], start=False, stop=True)
                    ht = wrk.tile([128, 512], F32R, tag=f"ht{hd}{hh}")
                    for ii in range(2):
                        ov = ht[:, ii * 256:(ii + 1) * 256]
                        iv = ps[:, ii * 256:(ii + 1) * 256]
                        bias = ab[:, hh * 256 + i0 + ii: hh * 256 + i0 + ii + 1]
                        if (hd, hh, ii) in EPI_ACT:
                            nc.scalar.activation(ov, iv, AF.Relu, bias=bias)
                        else:
                            nc.vector.tensor_scalar(ov, iv, bias, 0.0,
                                                    op0=A.add, op1=A.max)
                    hts[(hd, hh)] = ht
            # edge head 2nd layer -> [1, 512]
            pse = ps2.tile([1, 512], F32, tag="pe")
            for hh in range(2):
                nc.tensor.matmul(pse[:], hw2e[:, hh:hh + 1], hts[(0, hh)][:],
                                 start=(hh == 0), stop=(hh == 1))
            edge_sb = wrk.tile([1, 512], F32, tag="edge_sb")
            nc.scalar.activation(edge_sb[:], pse[:], AF.Identity,
                                 bias=hb2e[0:1, 0:1])
            for ii in range(2):
                nc.gpsimd.memset(edge_sb[0:1, ii * 256 + i0 + ii: ii * 256 + i0 + ii + 1], -8.0)
            nc.sync.dma_start(
                edge_d.rearrange("(np two) j -> np two j", two=2)[ip, :, :],
                edge_sb[:])
            # op head 2nd layer -> [8, 512]
            pso = ps2.tile([NOPS, 512], F32, tag="po")
            for hh in range(2):
                nc.tensor.matmul(pso[:], hw2o[:, hh * 8:(hh + 1) * 8], hts[(1, hh)][:],
                                 start=(hh == 0), stop=(hh == 1))
            op_sb = wrk.tile([NOPS, 512], F32, tag="op_sb")
            nc.vector.tensor_scalar(op_sb[:], pso[:], hb2o[:], None, op0=A.add)
            nc.sync.dma_start(
                op_d.rearrange("(np ii k) j -> np k ii j", ii=2, k=NOPS)[ip, :, :, :],
                op_sb[:])

    nc.compile()
    return nc


def _get_nc():
    if "nc" not in _NC_CACHE:
        _NC_CACHE["nc"] = _build()
    return _NC_CACHE["nc"]


TRACE = False
LAST_EXEC_NS = None


def kernel(**inputs):
    global LAST_EXEC_NS
    nc = _get_nc()
    w_names = [k for k in inputs if k not in ("obs", "latent", "query")]
    in_maps = []
    for c in range(N_CORES):
        b, half = c // 2, c % 2
        m = {k: np.ascontiguousarray(np.asarray(inputs[k], dtype=np.float32))
             for k in w_names}
        for k in ("obs", "latent", "query"):
            arr = np.asarray(inputs[k][b], dtype=np.float32)
            if half:
                arr = np.roll(arr, -NI, axis=0)
            m[k] = np.ascontiguousarray(arr)
        in_maps.append(m)

    res = run_bass_kernel_spmd(nc, in_maps, list(range(N_CORES)), trace=TRACE)
    LAST_EXEC_NS = res.exec_time_ns

    edge = np.empty((B, N, N), np.float32)
    op = np.empty((B, N, N, NOPS), np.float32)
    nxt = np.empty((B, N, 1), np.float32)
    for c in range(N_CORES):
        b, half = c // 2, c % 2
        r = res.results[c]
        e = r["edge_out"]                       # [128, 256] (j in rolled order)
        o = r["op_out"].reshape(NI, NOPS, N).transpose(0, 2, 1)  # [128, 256, 8]
        if half:
            e = np.roll(e, NI, axis=1)
            o = np.roll(o, NI, axis=1)
        rows = slice(half * NI, (half + 1) * NI)
        edge[b, rows, :] = e
        op[b, rows, :, :] = o
        nxt[b, rows, 0] = r["np_out"][:NI]
    return edge, op, nxt


if __name__ == "__main__":
    # quick shape sanity (no hardware)
    nc = _get_nc()
    print("built ok")
